# revision 1
# baseline (speedup 1.0000x reference)
import math
import sys

sys.path.insert(0, "/opt/trn_rl_repo")

import numpy as np

# Problem constants (hardcoded per spec)
NQ = 12
SEQ = 16
DD = 3
DIM = 1 << NQ
B_FULL = 2048
N_CORES = 8
B_LOC = B_FULL // N_CORES  # 256 samples per core
P = 128                    # partition tile (samples per tile)

_CACHE = {}


def _pbcast(bass, ap, prt):
    """Broadcast a DRAM tensor (no partition dim) across prt partitions."""
    return bass.AP(tensor=ap.tensor, offset=ap.offset, ap=[[0, prt]] + [list(d) for d in ap.ap])


def _bcast_free(bass, ap, pos, count):
    """Insert a [0, count] broadcast dim at position pos of an AP."""
    dims = [list(d) for d in ap.ap]
    dims.insert(pos, [0, count])
    return bass.AP(tensor=ap.tensor, offset=ap.offset, ap=dims)


def _halves(t, j, nq):
    """psi0/psi1 views of state tile [prt, 2^nq] for a gate on wire j."""
    s = 1 << (nq - 1 - j)
    if s == (1 << (nq - 1)):
        return t[:, :s], t[:, s:]
    v = t.rearrange("p (nb two s) -> p nb two s", two=2, s=s)
    return v[:, :, 0, :], v[:, :, 1, :]


def _mkv(bass, t, off, dims):
    """Manual strided view of a [prt, dim] tile (element offsets/steps)."""
    a = t[:, :]
    return bass.AP(
        tensor=a.tensor,
        offset=a.offset + off,
        ap=[list(a.ap[0])] + [list(d) for d in dims],
    )


def _contig_like(bass, scratch, off, ref_ap):
    """Contiguous view of `scratch` (from element `off`) shaped like ref_ap's free dims."""
    dims = [list(d) for d in ref_ap.ap[1:]]
    cont = []
    stride = 1
    for d in reversed(dims):
        cont.insert(0, [stride, d[1]])
        stride *= d[1]
    return _mkv(bass, scratch, off, cont), stride


def _gate_views(bass, Apl, Bpl, j, pending, nq):
    """List of (A0, A1, B0, B1) view-slices for the RY stage of a gate on wire j.

    A is read through the not-yet-materialized CNOT permutation `pending`
    (None | 'chain' = C(j, j+1 mod nq) | 'g1' = C(0,1) then C(1,2), j==1);
    B views are in the true basis, so the gate's full-state write
    materializes the permutation. Slicing by the true target-bit value
    keeps every view 2D/3D (walrus STT limit) with positive steps only.
    """
    dim = 1 << nq
    s = 1 << (nq - 1 - j)
    if pending is None:
        nb = dim // (2 * s)
        d0 = [[2 * s, nb], [1, s]]
        return [(
            _mkv(bass, Apl, 0, d0), _mkv(bass, Apl, s, d0),
            _mkv(bass, Bpl, 0, d0), _mkv(bass, Bpl, s, d0),
        )]
    if pending == "chain":
        if j < nq - 1:
            st = s // 2
            nb = dim // (2 * s)
            d = [[2 * s, nb], [1, st]]
            out = []
            for tau in (0, 1):
                out.append((
                    _mkv(bass, Apl, tau * st, d),
                    _mkv(bass, Apl, s + (1 - tau) * st, d),
                    _mkv(bass, Bpl, tau * st, d),
                    _mkv(bass, Bpl, s + tau * st, d),
                ))
            return out
        else:  # wrap: ctrl = nq-1, tgt = 0
            s0 = dim // 2
            d = [[2, s0 // 2]]
            out = []
            for tau in (0, 1):
                out.append((
                    _mkv(bass, Apl, tau * s0, d),
                    _mkv(bass, Apl, (1 - tau) * s0 + 1, d),
                    _mkv(bass, Bpl, tau * s0, d),
                    _mkv(bass, Bpl, tau * s0 + 1, d),
                ))
            return out
    assert pending == "g1" and j == 1
    s0, s1, s2 = dim // 2, dim // 4, dim // 8
    out = []
    for tau in (0, 1):
        out.append((
            _mkv(bass, Apl, tau * s2, [[s0 + s1, 2], [1, s2]]),
            _mkv(bass, Apl, s1 + (1 - tau) * s2, [[s0 - s1, 2], [1, s2]]),
            _mkv(bass, Bpl, tau * s2, [[s0, 2], [1, s2]]),
            _mkv(bass, Bpl, s1 + tau * s2, [[s0, 2], [1, s2]]),
        ))
    return out


def _ring_sign_wires(nq):
    """Per group g: wire set W' with parity_{W'}(s) == parity_{W_g}(ring(s))."""
    dim = 1 << nq
    s = np.arange(dim)
    for q in range(nq):
        c, t = (q, q + 1) if q < nq - 1 else (nq - 1, 0)
        s = s ^ (((s >> (nq - 1 - c)) & 1) << (nq - 1 - t))
    n3 = nq // 3
    out = []
    for g in range(3):
        par = np.zeros(dim, dtype=np.int64)
        for w in range(g * n3, (g + 1) * n3):
            par ^= (s >> (nq - 1 - w)) & 1
        sign = 1 - 2 * par
        wires = [w for w in range(nq) if sign[1 << (nq - 1 - w)] == -1]
        chk = np.zeros(dim, dtype=np.int64)
        for w in wires:
            chk ^= (np.arange(dim) >> (nq - 1 - w)) & 1
        assert np.array_equal(1 - 2 * chk, sign), "ring sign factorization failed"
        out.append(wires)
    return out


def build_program(nq=NQ, seq=SEQ, b_loc=B_LOC, n_cores=N_CORES, dtype16=False, repeat=1, mix=0):
    """Build and compile the per-core SPMD Bass program."""
    key = (nq, seq, b_loc, n_cores, dtype16, repeat, mix)
    if key in _CACHE:
        return _CACHE[key]

    import concourse.bass as bass
    import concourse.bacc as bacc
    import concourse.tile as tile
    from concourse import mybir

    FP = mybir.dt.float32
    ST = mybir.dt.float16 if dtype16 else mybir.dt.float32
    AF = mybir.ActivationFunctionType
    ALU = mybir.AluOpType
    AX = mybir.AxisListType.X

    dim = 1 << nq
    prt = min(P, b_loc)
    n_tiles = max(1, b_loc // prt)
    ng = seq * nq

    nc = bacc.Bacc("TRN2", target_bir_lowering=False, debug=False, num_devices=n_cores)
    x_ext = nc.dram_tensor("x", [b_loc, seq, DD], FP, kind="ExternalInput").ap()
    w_ext = nc.dram_tensor("w", [seq, nq, 2 * DD], FP, kind="ExternalInput").ap()
    b_ext = nc.dram_tensor("b", [seq, nq, 2], FP, kind="ExternalInput").ap()
    y_ext = nc.dram_tensor("y", [b_loc, 3], FP, kind="ExternalOutput").ap()

    with tile.TileContext(nc) as tc:
        with (
            tc.tile_pool(name="state", bufs=1) as st,
            tc.tile_pool(name="scal", bufs=1) as sc,
            tc.tile_pool(name="tmp", bufs=2) as tp,
            tc.tile_pool(name="scr", bufs=1) as scr,
        ):
            for tidx in range(n_tiles):
                stt = nc.vector.scalar_tensor_tensor
                tt = nc.vector.tensor_tensor

                # ---------- inputs ----------
                xt = sc.tile([prt, seq, DD], FP, tag=f"xt{tidx}")
                nc.sync.dma_start(out=xt, in_=x_ext[tidx * prt:(tidx + 1) * prt])
                wrep = sc.tile([prt, seq, nq, 2 * DD], FP, tag=f"wrep{tidx}")
                nc.sync.dma_start(out=wrep, in_=_pbcast(bass, w_ext, prt))
                brep = sc.tile([prt, seq, nq, 2], FP, tag=f"brep{tidx}")
                nc.sync.dma_start(out=brep, in_=_pbcast(bass, b_ext, prt))
                # halve once: angles enter as theta/2 everywhere
                nc.vector.tensor_scalar_mul(wrep, wrep, 0.5)
                nc.vector.tensor_scalar_mul(brep, brep, 0.5)

                # ---------- angles: h = 0.5*(x . W) + 0.5*bias ----------
                xrep = sc.tile([prt, seq, nq, DD], FP, tag=f"xrep{tidx}")
                for j in range(nq):
                    nc.vector.tensor_copy(xrep[:, :, j, :], xt)
                h = []
                for half in range(2):
                    prod = tp.tile([prt, seq, nq, DD], FP, tag="prod")
                    tt(prod, xrep, wrep[:, :, :, half * DD:(half + 1) * DD], ALU.mult)
                    hv = sc.tile([prt, seq, nq], FP, tag=f"h{half}_{tidx}")
                    nc.vector.tensor_reduce(hv, prod, axis=AX, op=ALU.add)
                    tt(hv, hv, brep[:, :, :, half], ALU.add)
                    h.append(hv)

                # ---------- per-gate scalars ----------
                # t = tan(h1), w = -tan(h2); per-step renorm g = prod_j cos(h1)cos(h2)
                pihalf = sc.tile([prt, 1], FP, tag=f"pihalf{tidx}")
                nc.vector.memset(pihalf, math.pi / 2)

                MAGIC = 1.5 * (2.0 ** 23)  # fp32 round-to-nearest-int trick
                TWO_PI = 2.0 * math.pi

                def trig(hv, tag):
                    # sin/cos of unbounded h via range reduction to [-pi, pi]
                    cv = sc.tile([prt, ng], FP, tag=f"c{tag}")
                    sv = tp.tile([prt, ng], FP, tag="sv")
                    hf = hv.rearrange("p a b -> p (a b)")
                    m = tp.tile([prt, ng], FP, tag="m")
                    nc.vector.tensor_scalar(m, hf, 1.0 / TWO_PI, None, ALU.mult)
                    k = tp.tile([prt, ng], FP, tag="k")
                    r = tp.tile([prt, ng], FP, tag="r")
                    # sin: r = m - round(m); x = 2*pi*r
                    nc.vector.tensor_scalar(k, m, MAGIC, MAGIC, ALU.add, ALU.subtract)
                    tt(r, m, k, ALU.subtract)
                    nc.vector.tensor_scalar(r, r, TWO_PI, None, ALU.mult)
                    nc.scalar.activation(sv, r, AF.Sin)
                    # cos: shift phase by +pi/2 (m + 0.25 turns)
                    mc = tp.tile([prt, ng], FP, tag="mc")
                    nc.vector.tensor_scalar(mc, m, 0.25, None, ALU.add)
                    nc.vector.tensor_scalar(k, mc, MAGIC, MAGIC, ALU.add, ALU.subtract)
                    tt(r, mc, k, ALU.subtract)
                    nc.vector.tensor_scalar(r, r, TWO_PI, None, ALU.mult)
                    nc.scalar.activation(cv, r, AF.Sin)
                    rcv = tp.tile([prt, ng], FP, tag="rcv")
                    nc.vector.reciprocal(rcv, cv)
                    dv = sc.tile([prt, ng], FP, tag=f"d{tag}")
                    tt(dv, sv, rcv, ALU.mult)
                    return cv, dv

                c1, t_ = trig(h[0], f"1_{tidx}")   # t_ = tan(h1)
                c2, wm = trig(h[1], f"2_{tidx}")   # wm = tan(h2) = -w
                if dtype16:
                    # bound per-gate dynamic range so fp16 transients stay finite
                    CLAMP = 240.0
                    for arr in (t_, wm):
                        nc.vector.tensor_scalar_min(arr, arr, CLAMP)
                        nc.vector.tensor_scalar_max(arr, arr, -CLAMP)
                tm = sc.tile([prt, ng], FP, tag=f"tm{tidx}")
                nc.vector.tensor_scalar_mul(tm, t_, -1.0)
                w_ = sc.tile([prt, ng], FP, tag=f"w{tidx}")
                nc.vector.tensor_scalar_mul(w_, wm, -1.0)
                gg = sc.tile([prt, ng], FP, tag=f"gg{tidx}")
                tt(gg, c1, c2, ALU.mult)
                # per-step product over the nq gates (pairwise tree; no mult-reduce)
                cur = gg.rearrange("p (a b) -> p a b", b=nq)
                n = nq
                lvl = 0
                while n > 1:
                    hn = n // 2
                    nxt = sc.tile([prt, seq, hn], FP, tag=f"gl{lvl}_{tidx}")
                    tt(nxt, cur[:, :, :hn], cur[:, :, hn:2 * hn], ALU.mult)
                    if n % 2:
                        tt(nxt[:, :, 0:1], nxt[:, :, 0:1], cur[:, :, n - 1:n], ALU.mult)
                    cur, n, lvl = nxt, hn, lvl + 1
                gcol = cur.rearrange("p a b -> p (a b)")

                for _rep in range(repeat):
                    # ---------- state init: step-0 product state ----------
                    # After step 0, psi = prod_j (e_j c_j, conj(e_j) s_j); in the
                    # deferred-scale basis v'_j = (1 + i*w_j, t_j*(1 - i*w_j)).
                    # Build by appending one wire per doubling (11 doublings
                    # replace step 0's 12 gates); ends in A (nq even).
                    Ar = st.tile([prt, dim], ST, tag=f"Ar{tidx}")
                    Ai = st.tile([prt, dim], ST, tag=f"Ai{tidx}")
                    Br = st.tile([prt, dim], ST, tag=f"Br{tidx}")
                    Bi = st.tile([prt, dim], ST, tag=f"Bi{tidx}")
                    nc.vector.memset(Br[:, 0:1], 1.0)
                    nc.vector.tensor_copy(Br[:, 1:2], t_[:, 0:1])
                    nc.vector.tensor_copy(Bi[:, 0:1], w_[:, 0:1])
                    tt(Bi[:, 1:2], t_[:, 0:1], wm[:, 0:1], ALU.mult)
                    Xr, Xi, Yr, Yi = Br, Bi, Ar, Ai
                    for jw in range(1, nq):
                        m = 1 << jw
                        wcj = w_[:, jw:jw + 1]
                        wmj = wm[:, jw:jw + 1]
                        tj = t_[:, jw:jw + 1]
                        Yvr = Yr[:, :2 * m].rearrange("p (m two) -> p m two", two=2)
                        Yvi = Yi[:, :2 * m].rearrange("p (m two) -> p m two", two=2)
                        stt(Yvr[:, :, 0], Xi[:, :m], wmj, Xr[:, :m], ALU.mult, ALU.add)
                        stt(Yvi[:, :, 0], Xr[:, :m], wcj, Xi[:, :m], ALU.mult, ALU.add)
                        stt(Xr[:, m:2 * m], Xi[:, :m], wcj, Xr[:, :m], ALU.mult, ALU.add)
                        stt(Xi[:, m:2 * m], Xr[:, :m], wmj, Xi[:, :m], ALU.mult, ALU.add)
                        nc.vector.tensor_scalar(Yvr[:, :, 1], Xr[:, m:2 * m], tj, None, ALU.mult)
                        nc.vector.tensor_scalar(Yvi[:, :, 1], Xi[:, m:2 * m], tj, None, ALU.mult)
                        Xr, Xi, Yr, Yi = Yr, Yi, Xr, Xi
                    assert Xr is Ar

                    # ---------- evolution ----------
                    # Step i's CNOT ring is folded into step i+1's gate reads:
                    # schedule [C01 C12] G1 [C23] G2 ... [C(10,11)] G10 [C(11,0)] G11 G0.
                    # The last step's ring is folded into the observable signs.
                    for i in range(seq):
                        if i == 0:
                            order = []  # step 0 applied via the product-state init
                        else:
                            order = [(1, "g1")] + [(j, "chain") for j in range(2, nq)] \
                                    + [(0, None)]
                        for (j, pend) in order:
                            k = i * nq + j
                            slr = _gate_views(bass, Ar, Br, j, pend, nq)
                            sli = _gate_views(bass, Ai, Bi, j, pend, nq)
                            tc_ = t_[:, k:k + 1]
                            tmc = tm[:, k:k + 1]
                            wc = w_[:, k:k + 1]
                            wmc = wm[:, k:k + 1]
                            gtt = nc.gpsimd.tensor_tensor
                            ns = len(slr)
                            mix_b1i = (mix >= 2 and k % 2 == 0)
                            S = scr.tile([prt, dim], FP, tag=f"mixS{tidx}") if mix else None
                            # RY (Givens, pending scale cos(h1)): u = [[1,-t],[t,1]] psi
                            # psi1-lane adds go ACT-mul + Pool-add; psi0-lane on DVE
                            soff = dim // 2
                            for si in range(ns):
                                A0r, A1r, B0r, B1r = slr[si]
                                A0i, A1i, B0i, B1i = sli[si]
                                stt(B0r, A1r, tmc, A0r, ALU.mult, ALU.add)
                                stt(B0i, A1i, tmc, A0i, ALU.mult, ALU.add)
                                stt(B1r, A0r, tc_, A1r, ALU.mult, ALU.add)
                                if mix_b1i:
                                    u3, sz = _contig_like(bass, S, soff, A0i)
                                    soff += sz
                                    nc.scalar.activation(u3, A0i, AF.Copy, bias=0.0, scale=tc_)
                                    gtt(B1i, u3, A1i, ALU.add)
                                else:
                                    stt(B1i, A0i, tc_, A1i, ALU.mult, ALU.add)
                            # RZ (pending scale cos(h2)): v0 = (1+iw)u0, v1 = (1-iw)u1
                            C0r, C1r = _halves(Ar, j, nq)
                            C0i, C1i = _halves(Ai, j, nq)
                            D0r, D1r = _halves(Br, j, nq)
                            D0i, D1i = _halves(Bi, j, nq)
                            stt(C0r, D0i, wmc, D0r, ALU.mult, ALU.add)
                            stt(C0i, D0r, wc, D0i, ALU.mult, ALU.add)
                            # C1r = (D1i*w) + D1r ; C1i = (D1r*wm) + D1i via ACT+Pool
                            if mix >= 1:
                                u1, _ = _contig_like(bass, S, 0, D1i)
                                nc.scalar.activation(u1, D1i, AF.Copy, bias=0.0, scale=wc)
                                gtt(C1r, u1, D1r, ALU.add)
                            else:
                                stt(C1r, D1i, wc, D1r, ALU.mult, ALU.add)
                            if mix >= 1:
                                u2, _ = _contig_like(bass, S, dim // 2, D1r)
                                nc.scalar.activation(u2, D1r, AF.Copy, bias=0.0, scale=wmc)
                                gtt(C1i, u2, D1i, ALU.add)
                            else:
                                stt(C1i, D1r, wmc, D1i, ALU.mult, ALU.add)
                            if dtype16:
                                # renorm each gate (ScalarE, hidden under DVE)
                                gk = gg[:, k:k + 1]
                                nc.scalar.activation(Ar, Ar, AF.Copy, bias=0.0, scale=gk)
                                nc.scalar.activation(Ai, Ai, AF.Copy, bias=0.0, scale=gk)
                        if not dtype16:
                            # renorm the deferred per-gate scales once per step
                            gc = gcol[:, i:i + 1]
                            nc.scalar.activation(Ar, Ar, AF.Copy, bias=0.0, scale=gc)
                            nc.scalar.activation(Ai, Ai, AF.Copy, bias=0.0, scale=gc)

                    # ---------- observables ----------
                    # p = |psi|^2 in fp32, then 3 signed halving trees
                    if dtype16:
                        Pr = st.tile([prt, dim], FP, tag="Pr")
                        Pi = st.tile([prt, dim], FP, tag="Pi")
                    else:
                        Pr, Pi = Br, Bi
                    nc.scalar.activation(Pr, Ar, AF.Square)
                    nc.scalar.activation(Pi, Ai, AF.Square)
                    tt(Pr, Pr, Pi, ALU.add)
                    out_t = tp.tile([prt, 3], FP, tag="out")
                    ring_wires = _ring_sign_wires(nq)
                    for grp in range(3):
                        wires = ring_wires[grp]
                        cur = Pr[:, :dim]
                        cur_n = dim
                        off = 0
                        for wi in wires:
                            s = 1 << (nq - 1 - wi)
                            a = cur_n // (2 * s)
                            v = cur.rearrange("p (a two s) -> p a two s", two=2, s=s, a=a)
                            nxt_n = cur_n // 2
                            nxt = Pi[:, off:off + nxt_n]
                            off += nxt_n
                            nv = nxt.rearrange("p (a s) -> p a s", a=a, s=s)
                            tt(nv, v[:, :, 0, :], v[:, :, 1, :], ALU.subtract)
                            cur = nxt
                            cur_n = nxt_n
                        ex = tp.tile([prt, 1], FP, tag="ex")
                        nc.vector.tensor_reduce(ex, cur, axis=AX, op=ALU.add)
                        # out = (exp + 1) / 2
                        nc.vector.tensor_scalar(
                            out_t[:, grp:grp + 1], ex, 0.5, 0.5, ALU.mult, ALU.add
                        )
                    nc.sync.dma_start(
                        out=y_ext[tidx * prt:(tidx + 1) * prt], in_=out_t
                    )

    nc.compile()
    _CACHE[key] = nc
    return nc


def kernel(x, weights, bias):
    from concourse.bass_utils import run_bass_kernel_spmd

    nc = build_program()
    in_maps = [
        {
            "x": np.ascontiguousarray(x[i * B_LOC:(i + 1) * B_LOC], dtype=np.float32),
            "w": np.ascontiguousarray(weights, dtype=np.float32),
            "b": np.ascontiguousarray(bias, dtype=np.float32),
        }
        for i in range(N_CORES)
    ]
    res = run_bass_kernel_spmd(nc, in_maps, list(range(N_CORES)))
    return np.concatenate([res.results[i]["y"] for i in range(N_CORES)], axis=0)

